# revision 21
# baseline (speedup 1.0000x reference)
"""Trainium2 Bass kernel for nn_CensoredLoss_Sub.

reference:
    out = outputs.reshape(B, T, D)                     # D = 2
    loss1 = targets[:, :, 0:1] * log((1 - out) + eps)
    loss2 = targets[:, :, 1:2] * log(out + eps)
    loss  = sum((loss1 + loss2) * weights[:, :, None], axis=(0, 1))  # (D,)
    return -loss / (B * T)

Strategy: pure data-parallel over B across 8 cores; per-core partial sums
are gathered and reduced on host (the (D,)=2-float all-reduce is trivial).

Key identity: for both d=0,1 the coefficient of log(1-o_d+eps) is w*t0 and
the coefficient of log(o_d+eps) is w*t1:
    loss_d = sum_pairs  (w*t0)*log(1-o_d+eps) + (w*t1)*log(o_d+eps)

Host-side layout (pure permutation, no arithmetic): per tile, o is
deinterleaved into [o0|o1] (f32 — it must stay f32: bf16 rounds o to
exactly 1.0 for ~0.2% of elements and 1-o+eps then underflows to 0 ->
Ln(0) = -inf, a catastrophic-cancellation path), and t/w are packed into
one [t0|t1|w] block that a single SWDGE DMA casts f32->bf16 in the DMA
datapath. Everything on-chip reads/writes contiguously (strided APs break
DVE 2x packing; ACT scattered writes run ~5x slow).

Per tile (F o-elems, FP=F/2 pairs per partition):
  ACT:  l1 = [log(1-o0+eps)|log(1-o1+eps)], l2 = [log(o0+eps)|log(o1+eps)]
        (4 ACTIVATEs, Ln, scale/bias fused, bf16)
  DVE:  X = w*t0, Y = w*t1; P1 = X_bcast*l1, P2 = Y_bcast*l2 (all 2x bf16)
  PE:   ones[128,1]^T @ P-chunks accumulated into psum_d0/psum_d1 [1,512]
        (partition-sum; column association is irrelevant — all summed later)
Final: ACT copies psum banks to SBUF, DMA [1,1024] out; host sums and
applies -1/(B*T).
"""

import numpy as np

B, T, D = 16384, 512, 2
N_CORES = 8
EPS = 1e-8
P = 128

FO = (B // N_CORES) * T * D // P  # o/t columns per partition = 16384
# DMA granularity is decoupled from compute granularity: big DMA tiles keep
# the HBM stream near peak; small compute sub-tiles keep dependency chains
# short and ride the stream closely. Host layout is deinterleaved at CHUNK
# granularity so every compute slice is contiguous.
CHUNK = 2048                      # compute sub-tile o-elems
DMA_TILES = [2048, 4096, 4096, 4096, 2048]
assert sum(DMA_TILES) == FO and all(t % CHUNK == 0 for t in DMA_TILES)
MM_N = 512                        # matmul moving free dim

_compiled = {}


def _build():
    import concourse.mybir as mybir
    from concourse import bacc
    from concourse.tile import TileContext

    f32 = mybir.dt.float32
    bf16 = mybir.dt.bfloat16
    Ln = mybir.ActivationFunctionType.Ln
    Copy = mybir.ActivationFunctionType.Copy

    nc = bacc.Bacc(
        "TRN2",
        target_bir_lowering=False,
        debug=False,
        num_devices=N_CORES,
    )
    o_d = nc.dram_tensor("o", [P, FO], f32, kind="ExternalInput").ap()
    tw_d = nc.dram_tensor("tw", [P, FO + FO // 2], f32, kind="ExternalInput").ap()
    acc_d = nc.dram_tensor("acc", [1, 2 * MM_N], f32, kind="ExternalOutput").ap()

    with TileContext(nc) as tc:
        with (
            tc.tile_pool(name="io", bufs=1) as io_pool,
            tc.tile_pool(name="mid", bufs=2) as mid_pool,
            tc.tile_pool(name="one", bufs=1) as one_pool,
            tc.tile_pool(name="ps", bufs=1, space="PSUM") as psum_pool,
        ):
            bias_eps = one_pool.tile([P, 1], f32)
            bias_1eps = one_pool.tile([P, 1], f32)
            ones = one_pool.tile([P, 1], bf16)
            res = one_pool.tile([1, 2 * MM_N], f32)
            nc.vector.memset(bias_eps[:], EPS)
            nc.vector.memset(bias_1eps[:], 1.0 + EPS)
            nc.vector.memset(ones[:], 1.0)
            psum0 = psum_pool.tile([1, MM_N], f32, tag="ps0")
            psum1 = psum_pool.tile([1, MM_N], f32, tag="ps1")
            psum = [psum0, psum1]
            dummy = one_pool.tile([P, 1], bf16)
            # warm the Ln table set while the first DMA is in flight
            nc.scalar.activation(dummy[:], bias_eps[:], Ln, bias=bias_1eps[:], scale=1.0)

            FP = CHUNK // 2
            FB = 3 * FP
            # Dedicated (bufs=1) buffers per DMA tile: no reuse deps, so every
            # DMA issues immediately; the single SWDGE queue is FIFO, so data
            # arrives strictly in tile order (o_i then blk_i) at full rate.
            subs = []
            o_off = 0
            tw_off = 0
            for ti, F in enumerate(DMA_TILES):
                ot = io_pool.tile([P, F], f32, tag=f"ot_i{ti}")
                blk = io_pool.tile([P, 3 * F // 2], bf16, tag=f"blk_i{ti}")
                nc.sync.dma_start(out=ot[:], in_=o_d[:, o_off : o_off + F])
                nc.gpsimd.dma_start(out=blk[:], in_=tw_d[:, tw_off : tw_off + 3 * F // 2])
                o_off += F
                tw_off += 3 * F // 2
                for k in range(F // CHUNK):
                    subs.append((ot, k * CHUNK, blk, k * FB))

            n_sub = len(subs)
            for i, (ot, oo, blk, bo) in enumerate(subs):
                osl = ot[:, oo : oo + CHUNK]
                tw = blk[:, bo : bo + FB].rearrange("p (c f) -> p c f", c=3)

                l1 = mid_pool.tile([P, CHUNK], bf16, tag="l1")
                l2 = mid_pool.tile([P, CHUNK], bf16, tag="l2")
                x = mid_pool.tile([P, FP], bf16, tag="x")
                y = mid_pool.tile([P, FP], bf16, tag="y")
                p1 = mid_pool.tile([P, CHUNK], bf16, tag="p1")
                p2 = mid_pool.tile([P, CHUNK], bf16, tag="p2")
                nc.vector.tensor_mul(x[:], tw[:, 2, :], tw[:, 0, :])
                nc.vector.tensor_mul(y[:], tw[:, 2, :], tw[:, 1, :])
                # one Ln per log-type over the whole [o0|o1] sub-tile
                nc.scalar.activation(l1[:], osl, Ln, bias=bias_1eps[:], scale=-1.0)
                nc.scalar.activation(l2[:], osl, Ln, bias=bias_eps[:], scale=1.0)
                for dd in range(2):
                    h = slice(dd * FP, (dd + 1) * FP)
                    nc.vector.tensor_mul(p1[:, h], x[:], l1[:, h])
                    nc.vector.tensor_mul(p2[:, h], y[:], l2[:, h])
                    for pi, prod in enumerate((p1, p2)):
                        for c in range(FP // MM_N):
                            first = i == 0 and pi == 0 and c == 0
                            last = (
                                i == n_sub - 1
                                and pi == 1
                                and c == FP // MM_N - 1
                            )
                            nc.tensor.matmul(
                                psum[dd][:],
                                ones[:],
                                prod[:, dd * FP + c * MM_N : dd * FP + (c + 1) * MM_N],
                                start=first,
                                stop=last,
                            )

            nc.scalar.activation(res[:, 0:MM_N], psum[0][:], Copy, bias=0.0, scale=1.0)
            nc.vector.tensor_copy(res[:, MM_N : 2 * MM_N], psum[1][:])
            nc.sync.dma_start(out=acc_d, in_=res[:])
    nc.compile()
    return nc


def _get_nc():
    if "nc" not in _compiled:
        _compiled["nc"] = _build()
    return _compiled["nc"]


def _deint(x2d):
    """[P, FO] interleaved -> per-CHUNK [d0-block | d1-block] layout."""
    n = FO // CHUNK
    v = x2d.reshape(P, n, CHUNK // 2, 2).transpose(0, 1, 3, 2)
    return np.ascontiguousarray(v).reshape(P, FO)


def _pack_tw(t2d, w2d):
    """Pack [P,FO] t (interleaved) + [P,FO/2] w into per-CHUNK [t0|t1|w]
    blocks -> [P, FO + FO//2]. Pure permutation/concatenation."""
    n = FO // CHUNK
    FP = CHUNK // 2
    tv = t2d.reshape(P, n, FP, 2).transpose(0, 1, 3, 2)  # [P, n, 2, FP]
    wv = w2d.reshape(P, n, 1, FP)
    blk = np.concatenate([tv, wv], axis=2)               # [P, n, 3, FP]
    return np.ascontiguousarray(blk).reshape(P, FO + FO // 2)


def make_in_maps(outputs, targets, weights):
    rows = B // N_CORES
    in_maps = []
    for c in range(N_CORES):
        sh = slice(c * rows, (c + 1) * rows)
        in_maps.append(
            {
                "o": _deint(np.ascontiguousarray(outputs[sh]).reshape(P, FO)),
                "tw": _pack_tw(
                    np.ascontiguousarray(targets[sh]).reshape(P, FO),
                    np.ascontiguousarray(weights[sh]).reshape(P, FO // 2),
                ),
            }
        )
    return in_maps


def run_raw(in_maps, **kw):
    from concourse import bass_utils

    nc = _get_nc()
    return bass_utils.run_bass_kernel_spmd(
        nc, in_maps, core_ids=list(range(N_CORES)), **kw
    )


def finish(results) -> np.ndarray:
    total = np.zeros(2, dtype=np.float64)
    for r in results:
        a = r["acc"].astype(np.float64).reshape(2, MM_N)
        total[0] += a[0].sum()
        total[1] += a[1].sum()
    return (-total / (B * T)).astype(np.float32)


def kernel(outputs: np.ndarray, targets: np.ndarray, weights: np.ndarray) -> np.ndarray:
    res = run_raw(make_in_maps(outputs, targets, weights))
    return finish(res.results)


# revision 22
# speedup vs baseline: 1.2489x; 1.2489x over previous
"""Trainium2 Bass kernel for nn_CensoredLoss_Sub.

reference:
    out = outputs.reshape(B, T, D)                     # D = 2
    loss1 = targets[:, :, 0:1] * log((1 - out) + eps)
    loss2 = targets[:, :, 1:2] * log(out + eps)
    loss  = sum((loss1 + loss2) * weights[:, :, None], axis=(0, 1))  # (D,)
    return -loss / (B * T)

Strategy: pure data-parallel over B across 8 cores; per-core partial sums
are gathered and reduced on host (the (D,)=2-float all-reduce is trivial).

Key identity: for both d=0,1 the coefficient of log(1-o_d+eps) is w*t0 and
the coefficient of log(o_d+eps) is w*t1:
    loss_d = sum_pairs  (w*t0)*log(1-o_d+eps) + (w*t1)*log(o_d+eps)

Host-side layout (pure permutation, no arithmetic): per tile, o is
deinterleaved into [o0|o1] (f32 — it must stay f32: bf16 rounds o to
exactly 1.0 for ~0.2% of elements and 1-o+eps then underflows to 0 ->
Ln(0) = -inf, a catastrophic-cancellation path), and t/w are packed into
one [t0|t1|w] block that a single SWDGE DMA casts f32->bf16 in the DMA
datapath. Everything on-chip reads/writes contiguously (strided APs break
DVE 2x packing; ACT scattered writes run ~5x slow).

Per tile (F o-elems, FP=F/2 pairs per partition):
  ACT:  l1 = [log(1-o0+eps)|log(1-o1+eps)], l2 = [log(o0+eps)|log(o1+eps)]
        (4 ACTIVATEs, Ln, scale/bias fused, bf16)
  DVE:  X = w*t0, Y = w*t1; P1 = X_bcast*l1, P2 = Y_bcast*l2 (all 2x bf16)
  PE:   ones[128,1]^T @ P-chunks accumulated into psum_d0/psum_d1 [1,512]
        (partition-sum; column association is irrelevant — all summed later)
Final: ACT copies psum banks to SBUF, DMA [1,1024] out; host sums and
applies -1/(B*T).
"""

import numpy as np

B, T, D = 16384, 512, 2
N_CORES = 8
EPS = 1e-8
P = 128

FO = (B // N_CORES) * T * D // P  # o/t columns per partition = 16384
# DMA granularity is decoupled from compute granularity: big DMA tiles keep
# the HBM stream near peak; small compute sub-tiles keep dependency chains
# short and ride the stream closely. Host layout is deinterleaved at CHUNK
# granularity so every compute slice is contiguous.
CHUNK = 2048                      # compute sub-tile o-elems
DMA_TILES = [4096, 4096, 4096, 4096]
assert sum(DMA_TILES) == FO and all(t % CHUNK == 0 for t in DMA_TILES)
MM_N = 512                        # matmul moving free dim

_compiled = {}


def _build():
    import concourse.mybir as mybir
    from concourse import bacc
    from concourse.tile import TileContext

    f32 = mybir.dt.float32
    bf16 = mybir.dt.bfloat16
    Ln = mybir.ActivationFunctionType.Ln
    Copy = mybir.ActivationFunctionType.Copy

    nc = bacc.Bacc(
        "TRN2",
        target_bir_lowering=False,
        debug=False,
        num_devices=N_CORES,
    )
    o_d = nc.dram_tensor("o", [P, FO], f32, kind="ExternalInput").ap()
    tw_d = nc.dram_tensor("tw", [P, FO + FO // 2], f32, kind="ExternalInput").ap()
    acc_d = nc.dram_tensor("acc", [1, 2 * MM_N], f32, kind="ExternalOutput").ap()

    with TileContext(nc) as tc:
        with (
            tc.tile_pool(name="io", bufs=2) as io_pool,
            tc.tile_pool(name="mid", bufs=3) as mid_pool,
            tc.tile_pool(name="one", bufs=1) as one_pool,
            tc.tile_pool(name="ps", bufs=1, space="PSUM") as psum_pool,
        ):
            bias_eps = one_pool.tile([P, 1], f32)
            bias_1eps = one_pool.tile([P, 1], f32)
            ones = one_pool.tile([P, 1], bf16)
            res = one_pool.tile([1, 2 * MM_N], f32)
            nc.vector.memset(bias_eps[:], EPS)
            nc.vector.memset(bias_1eps[:], 1.0 + EPS)
            nc.vector.memset(ones[:], 1.0)
            psum0 = psum_pool.tile([1, MM_N], f32, tag="ps0")
            psum1 = psum_pool.tile([1, MM_N], f32, tag="ps1")
            psum = [psum0, psum1]
            dummy = one_pool.tile([P, 1], bf16)
            # warm the Ln table set while the first DMA is in flight
            nc.scalar.activation(dummy[:], bias_eps[:], Ln, bias=bias_1eps[:], scale=1.0)

            FP = CHUNK // 2
            FB = 3 * FP
            # One HWDGE queue for everything: FIFO per-tile order (o_i, tw_i)
            # keeps arrival strictly sequential at full single-queue rate
            # (two concurrent queues measured ~18% slower). t/w stay f32 in
            # SBUF; the x/y multiplies emit bf16 directly (free cast).
            # The last tile's DMAs are split per CHUNK so the tail
            # dependencies clear as early as possible.
            subs = []
            o_off = 0
            tw_off = 0
            for ti, F in enumerate(DMA_TILES):
                ot = io_pool.tile([P, F], f32, tag="ot")
                blk = io_pool.tile([P, 3 * F // 2], f32, tag="blk")
                n_split = F // CHUNK if ti == len(DMA_TILES) - 1 else 1
                fs = F // n_split
                bs = 3 * fs // 2
                for k in range(n_split):
                    nc.sync.dma_start(
                        out=ot[:, k * fs : (k + 1) * fs],
                        in_=o_d[:, o_off + k * fs : o_off + (k + 1) * fs],
                    )
                    nc.sync.dma_start(
                        out=blk[:, k * bs : (k + 1) * bs],
                        in_=tw_d[:, tw_off + k * bs : tw_off + (k + 1) * bs],
                    )
                o_off += F
                tw_off += 3 * F // 2
                for k in range(F // CHUNK):
                    subs.append((ot, k * CHUNK, blk, k * FB))

            n_sub = len(subs)
            for i, (ot, oo, blk, bo) in enumerate(subs):
                osl = ot[:, oo : oo + CHUNK]
                tw = blk[:, bo : bo + FB].rearrange("p (c f) -> p c f", c=3)

                l1 = mid_pool.tile([P, CHUNK], bf16, tag="l1")
                l2 = mid_pool.tile([P, CHUNK], bf16, tag="l2")
                x = mid_pool.tile([P, FP], bf16, tag="x")
                y = mid_pool.tile([P, FP], bf16, tag="y")
                p1 = mid_pool.tile([P, CHUNK], bf16, tag="p1")
                p2 = mid_pool.tile([P, CHUNK], bf16, tag="p2")
                nc.vector.tensor_mul(x[:], tw[:, 2, :], tw[:, 0, :])
                nc.vector.tensor_mul(y[:], tw[:, 2, :], tw[:, 1, :])
                # one Ln per log-type over the whole [o0|o1] sub-tile
                nc.scalar.activation(l1[:], osl, Ln, bias=bias_1eps[:], scale=-1.0)
                nc.scalar.activation(l2[:], osl, Ln, bias=bias_eps[:], scale=1.0)
                for dd in range(2):
                    h = slice(dd * FP, (dd + 1) * FP)
                    nc.vector.tensor_mul(p1[:, h], x[:], l1[:, h])
                    nc.vector.tensor_mul(p2[:, h], y[:], l2[:, h])
                    for pi, prod in enumerate((p1, p2)):
                        for c in range(FP // MM_N):
                            first = i == 0 and pi == 0 and c == 0
                            last = (
                                i == n_sub - 1
                                and pi == 1
                                and c == FP // MM_N - 1
                            )
                            nc.tensor.matmul(
                                psum[dd][:],
                                ones[:],
                                prod[:, dd * FP + c * MM_N : dd * FP + (c + 1) * MM_N],
                                start=first,
                                stop=last,
                            )

            nc.scalar.activation(res[:, 0:MM_N], psum[0][:], Copy, bias=0.0, scale=1.0)
            nc.vector.tensor_copy(res[:, MM_N : 2 * MM_N], psum[1][:])
            nc.sync.dma_start(out=acc_d, in_=res[:])
    nc.compile()
    return nc


def _get_nc():
    if "nc" not in _compiled:
        _compiled["nc"] = _build()
    return _compiled["nc"]


def _deint(x2d):
    """[P, FO] interleaved -> per-CHUNK [d0-block | d1-block] layout."""
    n = FO // CHUNK
    v = x2d.reshape(P, n, CHUNK // 2, 2).transpose(0, 1, 3, 2)
    return np.ascontiguousarray(v).reshape(P, FO)


def _pack_tw(t2d, w2d):
    """Pack [P,FO] t (interleaved) + [P,FO/2] w into per-CHUNK [t0|t1|w]
    blocks -> [P, FO + FO//2]. Pure permutation/concatenation."""
    n = FO // CHUNK
    FP = CHUNK // 2
    tv = t2d.reshape(P, n, FP, 2).transpose(0, 1, 3, 2)  # [P, n, 2, FP]
    wv = w2d.reshape(P, n, 1, FP)
    blk = np.concatenate([tv, wv], axis=2)               # [P, n, 3, FP]
    return np.ascontiguousarray(blk).reshape(P, FO + FO // 2)


def make_in_maps(outputs, targets, weights):
    rows = B // N_CORES
    in_maps = []
    for c in range(N_CORES):
        sh = slice(c * rows, (c + 1) * rows)
        in_maps.append(
            {
                "o": _deint(np.ascontiguousarray(outputs[sh]).reshape(P, FO)),
                "tw": _pack_tw(
                    np.ascontiguousarray(targets[sh]).reshape(P, FO),
                    np.ascontiguousarray(weights[sh]).reshape(P, FO // 2),
                ),
            }
        )
    return in_maps


def run_raw(in_maps, **kw):
    from concourse import bass_utils

    nc = _get_nc()
    return bass_utils.run_bass_kernel_spmd(
        nc, in_maps, core_ids=list(range(N_CORES)), **kw
    )


def finish(results) -> np.ndarray:
    total = np.zeros(2, dtype=np.float64)
    for r in results:
        a = r["acc"].astype(np.float64).reshape(2, MM_N)
        total[0] += a[0].sum()
        total[1] += a[1].sum()
    return (-total / (B * T)).astype(np.float32)


def kernel(outputs: np.ndarray, targets: np.ndarray, weights: np.ndarray) -> np.ndarray:
    res = run_raw(make_in_maps(outputs, targets, weights))
    return finish(res.results)
